# revision 2
# baseline (speedup 1.0000x reference)
"""Trainium2 Bass kernel for nn_Attention_52166672777669 (sparse_attention).

Math (reference):
    q  = LN(qx; g_q, b_q) @ wq.T                        # [256, 512]
    k  = LN(kx; g_k, b_k) @ wk.T                        # [256, 512, 512]
    S[q, kb, n] = (q[q] . k[kb, n]) / sqrt(512)         # masked, softmax over n
    out[q, kb, :] = sum_n P[q, kb, n] * kx[kb, n, :]    # [256, 256, 512]

Algebraic restructuring (exact up to fp rounding):
  S.T[n,q] = (r_n * kx[kb]) @ Qg.T   per key-batch kb, where
  Qg = (1/sqrt(C)) * g_k * (LN(qx) @ wq.T @ wk), row-centered
  (centering folds the k-side LN mean term; q-only additive terms are
  softmax-invariant and dropped; r_n = rsqrt(var_c kx[n,:] + eps)).
  Qg, r_n, and the r-scaled transposed kx stream are computed on the
  host (input marshaling scale, one pass over kx).

Device work per key batch (T = #active 128-row key tiles, mask-skipped):
  QK: 4 fp16 matmuls per tile (kxt stationary, Qg.T streaming), tiles 0+1
      share one PSUM bank so one 512-col Exp covers both (no bias там --
      every key batch here has >=256 valid keys); tiles >=2 exp with a
      per-row mask-bias column.  AV fp16 + ones-column denominators,
      divide (Pool/DVE), packed store.
  kx ships twice (host-transposed + natural), both streams trimmed to the
  per-slot valid length.

Sharding: Bk across 8 cores; batches sorted by valid length and dealt
round-robin so every core runs the same per-slot tile count T_j.
No collectives.
"""

import os
import sys

import numpy as np

for _p in ("/opt/trn_rl_repo",):
    if _p not in sys.path and os.path.isdir(_p):
        sys.path.insert(0, _p)

Bq, Bk, Nk, C = 256, 256, 512, 512
NCORES = 8
BKPC = Bk // NCORES  # key-batch slots per core
EPS = 1e-5
MASK_NEG = -100000.0

_cache = {}


def _lengths_from_mask(mask: np.ndarray) -> np.ndarray:
    """Per key-batch: last unmasked index + 1 (tiles needed = ceil(L/128))."""
    valid = ~np.asarray(mask, bool)
    any_valid = valid.any(axis=1)
    last = np.where(any_valid, Nk - 1 - np.argmax(valid[:, ::-1], axis=1), 0)
    return (last + 1).astype(np.int64)


def _plan(mask: np.ndarray):
    """Sort batches by valid length, deal round-robin across cores so the
    (shared) program's per-slot plan is tight for every core.

    Returns (perm, plan) where plan[j] = (T_j, nr_j, pair_j):
      T_j   tiles to compute, nr_j  valid rows in the last tile (1..128),
      pair_j  True when tiles 0,1 are fully valid for every batch in slot.
    """
    mask = np.asarray(mask, bool)
    L = _lengths_from_mask(mask)
    order = np.argsort(L, kind="stable")
    perm = order.reshape(BKPC, NCORES)  # perm[j, i] = global kb of core i, slot j
    plan = []
    for j in range(BKPC):
        bs = perm[j]
        Lmax = int(L[bs].max())
        T = max(1, -(-Lmax // 128))
        nr = Lmax - (T - 1) * 128  # 1..128
        pair = (
            T >= 2
            and not mask[bs, : 2 * 128].any()
            and (T >= 3 or nr == 128)  # paired tiles must be full height
        )
        plan.append((T, int(nr), bool(pair)))
    return perm, tuple(plan)


def _build_nc(plan):
    from contextlib import ExitStack

    import concourse.bacc as bacc
    import concourse.bass as bass
    import concourse.mybir as mybir
    import concourse.tile as tile

    f16 = mybir.dt.float16
    f32 = mybir.dt.float32
    ts = bass.ts
    AF = mybir.ActivationFunctionType
    ALU = mybir.AluOpType

    nc = bacc.Bacc()

    # [p][ci][q]: Qg.T fp16, c = ci*128 + p
    qgT_d = nc.declare_dram_parameter("qgT", [128, 4 * Bq], f16, isOutput=False)
    ones_d = nc.declare_dram_parameter("ones", [128, 1], f16, isOutput=False)
    # per slot j, tile t: mask bias column (0 or MASK_NEG)
    rb_d = nc.declare_dram_parameter("rb", [128, BKPC * 4], f32, isOutput=False)
    # [b][p][t*C + c] fp16, n = t*128 + p
    kxn_d = nc.declare_dram_parameter("kxn", [BKPC, 128, 4 * C], f16, isOutput=False)
    # r-scaled transposed kx: per slot: (T-1) full tiles [ci][n] then a
    # trimmed last tile [ci][0:nr]; c = ci*128 + p
    kxt_d = nc.declare_dram_parameter("kxt", [BKPC, 128, 4 * 512], f16, isOutput=False)
    # packed output: [b][p][mt][c] -> host unpacks to [b, mt*128+p, c]
    out_d = nc.declare_dram_parameter("out", [BKPC, 128, 2 * C], f16, isOutput=True)

    with tile.TileContext(nc) as tc, ExitStack() as ctx:
        consts = ctx.enter_context(tc.tile_pool(name="consts", bufs=1))
        work = ctx.enter_context(tc.tile_pool(name="work", bufs=2))
        ps = ctx.enter_context(tc.tile_pool(name="ps", bufs=1, space="PSUM"))

        qgT = consts.tile([128, 4, Bq], f16)
        nc.gpsimd.dma_start(qgT[:, :, :], qgT_d[:, :])
        ones_col = consts.tile([128, 1], f16)
        nc.gpsimd.dma_start(ones_col[:], ones_d[:, :])
        rb = consts.tile([128, BKPC * 4], f32)
        nc.gpsimd.dma_start(rb[:], rb_d[:, :])

        # single ACT LUT load for the whole kernel: one dummy Exp up front
        dummy = work.tile([128, 1], f16, tag="dummy")
        nc.scalar.activation(dummy[:], rb[:, 0:1], AF.Exp, bias=rb[:, 0:1], scale=0.0)

        KB, TB = 4, 4  # kxn / kxt pool depths

        for g in range(BKPC):
            T, nr, pair = plan[g]
            kxn = work.tile([128, 4 * C], f16, tag="kxn", bufs=KB)
            kxt = work.tile([128, 4, 4, 128], f16, tag="kxt", bufs=TB)
            nc.sync.dma_start(kxt[:, 0:T, :, :], kxt_d[g, :, 0 : T * 512])
            nc.sync.dma_start(kxn[:, 0 : T * C], kxn_d[g, :, 0 : T * C])

            # ---- scores S.T[n, q] ; exp -> pT fp16 ----
            # per-tile valid row count: ops slice to kh rows so trimmed
            # loads are never read beyond what the DMA wrote
            kh = [128] * (T - 1) + [nr]
            pTs = [None] * T

            def qk_chain(t, psum_view, first_in_bank):
                for ci in range(4):
                    nc.tensor.matmul(
                        psum_view,
                        kxt[:, t, ci, 0 : kh[t]],
                        qgT[:, ci, :],
                        start=(ci == 0 and first_in_bank),
                        stop=(ci == 3),
                        skip_group_check=not first_in_bank,
                    )

            t0 = 0
            if pair:
                psa = ps.tile([128, 2 * Bq], f32, tag="psa", bufs=2)
                qk_chain(0, psa[:, 0:Bq], True)
                qk_chain(1, psa[:, Bq : 2 * Bq], False)
                pe = work.tile([128, 2 * Bq], f16, tag="pTp", bufs=2)
                nc.scalar.activation(pe[:], psa[:], AF.Exp)
                pTs[0] = pe[:, 0:Bq]
                pTs[1] = pe[:, Bq : 2 * Bq]
                t0 = 2
            for t in range(t0, T):
                h = kh[t]
                psb = ps.tile([128, Bq], f32, tag="psb", bufs=2)
                qk_chain(t, psb[0:h, :], True)
                pe = work.tile([128, Bq], f16, tag=f"pT{t}", bufs=2)
                col = g * 4 + t
                nc.scalar.activation(
                    pe[0:h, :], psb[0:h, :], AF.Exp, bias=rb[0:h, col : col + 1]
                )
                pTs[t] = pe[:]

            # ---- denom + AV interleaved (shared lhsT per (mt, t)) ----
            psd = ps.tile([128, 2], f32, tag="psd", bufs=1)
            rd = work.tile([128, 2], f32, tag="rd", bufs=2)
            osb = work.tile([128, 2 * C], f16, tag="osb", bufs=3)
            for mt in range(2):
                pso = ps.tile([128, C], f32, tag="pso", bufs=3)
                for t in range(T):
                    h = kh[t]
                    lhs = pTs[t][0:h, ts(mt, 128)]
                    nc.tensor.matmul(
                        psd[:, mt : mt + 1],
                        lhs,
                        ones_col[0:h, :],
                        start=(t == 0),
                        stop=(t == T - 1),
                    )
                    nc.tensor.matmul(
                        pso[:],
                        lhs,
                        kxn[0:h, ts(t, C)],
                        start=(t == 0),
                        stop=(t == T - 1),
                    )
                nc.vector.reciprocal(rd[:, mt : mt + 1], psd[:, mt : mt + 1])
                if mt == 0:
                    nc.scalar.mul(osb[:, ts(mt, C)], pso[:], rd[:, mt : mt + 1])
                else:
                    nc.vector.tensor_scalar(
                        osb[:, ts(mt, C)],
                        pso[:],
                        rd[:, mt : mt + 1],
                        None,
                        op0=ALU.mult,
                    )
            nc.scalar.dma_start(out_d[g, :, :], osb[:])

    nc.compile()
    return nc


def _prep_host(qx, kx, key_padding_mask, ln_q_g, ln_q_b, ln_k_g, ln_k_b, wq, wk):
    f32 = np.float32
    mask = np.asarray(key_padding_mask, bool)
    perm, plan = _plan(mask)

    # ---- Qg on host (exact restructure; see module docstring) ----
    qx32 = np.asarray(qx, f32).reshape(Bq, C)
    m = qx32.mean(axis=1, keepdims=True)
    v = ((qx32 - m) ** 2).mean(axis=1, keepdims=True)
    ln = (qx32 - m) / np.sqrt(v + EPS) * np.asarray(ln_q_g, f32)[None, :] + np.asarray(
        ln_q_b, f32
    )[None, :]
    qvec = ln.astype(np.float16).astype(f32) @ np.asarray(wq, f32).T
    qhat = qvec @ np.asarray(wk, f32)
    qg = qhat * (np.asarray(ln_k_g, f32) * (C ** -0.5))[None, :]
    qg = qg - qg.mean(axis=1, keepdims=True)  # fold k-side LN mean term
    qgT = np.ascontiguousarray(qg.T).astype(np.float16)  # [c, q]
    qgT_p = np.ascontiguousarray(
        qgT.reshape(4, 128, Bq).transpose(1, 0, 2).reshape(128, 4 * Bq)
    )

    # ---- per-row LN stats of kx on host; fold rsqrt(var) into kxt ----
    kx32 = np.asarray(kx, f32)  # [Bk, Nk, C]
    mk = kx32.mean(axis=-1, keepdims=True)
    vk = ((kx32 - mk) ** 2).mean(axis=-1)  # [Bk, Nk]
    r = 1.0 / np.sqrt(vk + EPS)
    bias = np.where(mask, MASK_NEG, 0.0).astype(f32)  # [Bk, Nk]

    kx16 = np.asarray(kx, np.float16)
    kxt_all = (kx32 * r[:, :, None]).astype(np.float16)  # r-scaled, [kb, n, c]
    ones = np.ones((128, 1), np.float16)
    in_maps = []
    for i in range(NCORES):
        batches = perm[:, i]
        kxs = kx16[batches]  # [BKPC, Nk, C]
        kxn = np.ascontiguousarray(
            kxs.reshape(BKPC, 4, 128, C).transpose(0, 2, 1, 3).reshape(BKPC, 128, 4 * C)
        )
        # kxt: [b][p][t][ci][n] = r*kx[b, t*128+n, ci*128+p], last tile
        # packed trimmed: cols (T-1)*512 + ci*nr + n
        a = kxt_all[batches].transpose(0, 2, 1)  # [b, c, n]
        full = (
            a.reshape(BKPC, 4, 128, 4, 128)  # [b, ci, p, t, n]
            .transpose(0, 2, 3, 1, 4)  # [b, p, t, ci, n]
            .reshape(BKPC, 128, 4 * 512)
        )
        kxt = np.ascontiguousarray(full)
        rbv = np.zeros((128, BKPC * 4), f32)
        bslab = bias[batches]  # [BKPC, Nk]
        for j in range(BKPC):
            rbv[:, j * 4 : j * 4 + 4] = bslab[j].reshape(4, 128).T
        in_maps.append(
            dict(
                qgT=qgT_p,
                ones=ones,
                rb=np.ascontiguousarray(rbv),
                kxn=kxn,
                kxt=np.ascontiguousarray(kxt),
            )
        )
    return in_maps, perm, plan


def _get_nc(plan):
    if _cache.get("plan") != plan:
        _cache["nc"] = _build_nc(plan)
        _cache["plan"] = plan
    return _cache["nc"]


def kernel(**inputs) -> np.ndarray:
    from concourse.bass_utils import run_bass_kernel_spmd

    in_maps, perm, plan = _prep_host(**inputs)
    nc = _get_nc(plan)
    res = run_bass_kernel_spmd(nc, in_maps, list(range(NCORES)))
    full = np.empty((Bq, Bk, C), np.float16)
    for i in range(NCORES):
        o = res.results[i]["out"]  # [BKPC, 128, 2C] packed
        o = o.reshape(BKPC, 128, 2, C).transpose(0, 2, 1, 3).reshape(BKPC, Bq, C)
        full[:, perm[:, i], :] = o.transpose(1, 0, 2)
    return np.ascontiguousarray(full)


# revision 3
# speedup vs baseline: 1.0148x; 1.0148x over previous
"""Trainium2 Bass kernel for nn_Attention_52166672777669 (sparse_attention).

Math (reference):
    q  = LN(qx; g_q, b_q) @ wq.T                        # [256, 512]
    k  = LN(kx; g_k, b_k) @ wk.T                        # [256, 512, 512]
    S[q, kb, n] = (q[q] . k[kb, n]) / sqrt(512)         # masked, softmax over n
    out[q, kb, :] = sum_n P[q, kb, n] * kx[kb, n, :]    # [256, 256, 512]

Algebraic restructuring (exact up to fp rounding):
  S.T[n,q] = (r_n * kx[kb]) @ Qg.T   per key-batch kb, where
  Qg = (1/sqrt(C)) * g_k * (LN(qx) @ wq.T @ wk), row-centered
  (centering folds the k-side LN mean term; q-only additive terms are
  softmax-invariant and dropped; r_n = rsqrt(var_c kx[n,:] + eps)).
  Qg, r_n, and the r-scaled transposed kx stream are computed on the
  host (input marshaling scale, one pass over kx).

Device work per key batch (T = per-slot active 128-row key tiles; fully
masked tiles are skipped entirely):
  QK: 4 fp16 matmuls per tile (kxt stationary, Qg.T streaming); tiles 0+1
      share one PSUM bank so one 512-col Exp covers both (every key batch
      here has >=256 valid keys); later tiles exp with a per-row
      mask-bias column.  AV fp16 + ones-column denominators, divide
      (ACT + DVE), packed store.
  kx ships twice (host-transposed r-scaled + natural layout) as full
  contiguous per-partition chunks -- big DMA packets beat byte trims.

Sharding: Bk across 8 cores; batches sorted by valid length and dealt
round-robin so every core runs the same per-slot plan. No collectives.
"""

import os
import sys

import numpy as np

for _p in ("/opt/trn_rl_repo",):
    if _p not in sys.path and os.path.isdir(_p):
        sys.path.insert(0, _p)

Bq, Bk, Nk, C = 256, 256, 512, 512
NCORES = 8
BKPC = Bk // NCORES  # key-batch slots per core
EPS = 1e-5
MASK_NEG = -100000.0

_cache = {}


def _lengths_from_mask(mask: np.ndarray) -> np.ndarray:
    """Per key-batch: last unmasked index + 1 (tiles needed = ceil(L/128))."""
    valid = ~np.asarray(mask, bool)
    any_valid = valid.any(axis=1)
    last = np.where(any_valid, Nk - 1 - np.argmax(valid[:, ::-1], axis=1), 0)
    return (last + 1).astype(np.int64)


def _plan(mask: np.ndarray):
    """Sort batches by valid length, deal round-robin across cores so the
    (shared) program's per-slot plan is tight for every core.

    Returns (perm, plan) where plan[j] = (T_j, nr_j, pair_j):
      T_j   tiles to compute, nr_j  valid rows in the last tile (1..128),
      pair_j  True when tiles 0,1 are fully valid for every batch in slot.
    """
    mask = np.asarray(mask, bool)
    L = _lengths_from_mask(mask)
    order = np.argsort(L, kind="stable")
    perm = order.reshape(BKPC, NCORES)  # perm[j, i] = global kb of core i, slot j
    plan = []
    for j in range(BKPC):
        bs = perm[j]
        Lmax = int(L[bs].max())
        T = max(1, -(-Lmax // 128))
        nr = Lmax - (T - 1) * 128  # 1..128
        pair = (
            T >= 2
            and not mask[bs, : 2 * 128].any()
            and (T >= 3 or nr == 128)  # paired tiles must be full height
        )
        plan.append((T, int(nr), bool(pair)))
    return perm, tuple(plan)


def _build_nc(plan):
    from contextlib import ExitStack

    import concourse.bacc as bacc
    import concourse.bass as bass
    import concourse.mybir as mybir
    import concourse.tile as tile

    f16 = mybir.dt.float16
    f32 = mybir.dt.float32
    ts = bass.ts
    AF = mybir.ActivationFunctionType
    ALU = mybir.AluOpType

    nc = bacc.Bacc()

    # [p][ci][q]: Qg.T fp16, c = ci*128 + p
    qgT_d = nc.declare_dram_parameter("qgT", [128, 4 * Bq], f16, isOutput=False)
    ones_d = nc.declare_dram_parameter("ones", [128, 1], f16, isOutput=False)
    # per slot j, tile t: mask bias column (0 or MASK_NEG)
    rb_d = nc.declare_dram_parameter("rb", [128, BKPC * 4], f32, isOutput=False)
    # [b][p][t*C + c] fp16, n = t*128 + p
    kxn_d = nc.declare_dram_parameter("kxn", [BKPC, 128, 4 * C], f16, isOutput=False)
    # r-scaled transposed kx: per slot: (T-1) full tiles [ci][n] then a
    # trimmed last tile [ci][0:nr]; c = ci*128 + p
    kxt_d = nc.declare_dram_parameter("kxt", [BKPC, 128, 4 * 512], f16, isOutput=False)
    # packed output: [b][p][mt][c] -> host unpacks to [b, mt*128+p, c]
    out_d = nc.declare_dram_parameter("out", [BKPC, 128, 2 * C], f16, isOutput=True)

    with tile.TileContext(nc) as tc, ExitStack() as ctx:
        consts = ctx.enter_context(tc.tile_pool(name="consts", bufs=1))
        work = ctx.enter_context(tc.tile_pool(name="work", bufs=2))
        ps = ctx.enter_context(tc.tile_pool(name="ps", bufs=1, space="PSUM"))

        qgT = consts.tile([128, 4, Bq], f16)
        nc.gpsimd.dma_start(qgT[:, :, :], qgT_d[:, :])
        ones_col = consts.tile([128, 1], f16)
        nc.gpsimd.dma_start(ones_col[:], ones_d[:, :])
        rb = consts.tile([128, BKPC * 4], f32)
        nc.gpsimd.dma_start(rb[:], rb_d[:, :])

        # single ACT LUT load for the whole kernel: one dummy Exp up front
        dummy = work.tile([128, 1], f16, tag="dummy")
        nc.scalar.activation(dummy[:], rb[:, 0:1], AF.Exp, bias=rb[:, 0:1], scale=0.0)

        KB, TB = 4, 4  # kxn / kxt pool depths

        for g in range(BKPC):
            T, nr, pair = plan[g]
            kxn = work.tile([128, 4 * C], f16, tag="kxn", bufs=KB)
            kxt = work.tile([128, 4, 4, 128], f16, tag="kxt", bufs=TB)
            nc.sync.dma_start(kxt[:, 0:T, :, :], kxt_d[g, :, 0 : T * 512])
            nc.sync.dma_start(kxn[:, 0 : T * C], kxn_d[g, :, 0 : T * C])

            # ---- scores S.T[n, q] ; exp -> pT fp16 ----
            # per-tile valid row count: ops slice to kh rows so trimmed
            # loads are never read beyond what the DMA wrote
            kh = [128] * (T - 1) + [nr]
            pTs = [None] * T

            def qk_chain(t, psum_view, first_in_bank):
                for ci in range(4):
                    nc.tensor.matmul(
                        psum_view,
                        kxt[:, t, ci, 0 : kh[t]],
                        qgT[:, ci, :],
                        start=(ci == 0 and first_in_bank),
                        stop=(ci == 3),
                        skip_group_check=not first_in_bank,
                    )

            t0 = 0
            if pair:
                psa = ps.tile([128, 2 * Bq], f32, tag="psa", bufs=2)
                qk_chain(0, psa[:, 0:Bq], True)
                qk_chain(1, psa[:, Bq : 2 * Bq], False)
                pe = work.tile([128, 2 * Bq], f16, tag="pTp", bufs=2)
                nc.scalar.activation(pe[:], psa[:], AF.Exp)
                pTs[0] = pe[:, 0:Bq]
                pTs[1] = pe[:, Bq : 2 * Bq]
                t0 = 2
            for t in range(t0, T):
                h = kh[t]
                psb = ps.tile([128, Bq], f32, tag="psb", bufs=2)
                qk_chain(t, psb[0:h, :], True)
                pe = work.tile([128, Bq], f16, tag=f"pT{t}", bufs=2)
                col = g * 4 + t
                nc.scalar.activation(
                    pe[0:h, :], psb[0:h, :], AF.Exp, bias=rb[0:h, col : col + 1]
                )
                pTs[t] = pe[:]

            # ---- denom + AV interleaved (shared lhsT per (mt, t)) ----
            psd = ps.tile([128, 2], f32, tag="psd", bufs=1)
            rd = work.tile([128, 2], f32, tag="rd", bufs=2)
            osb = work.tile([128, 2 * C], f16, tag="osb", bufs=3)
            for mt in range(2):
                pso = ps.tile([128, C], f32, tag="pso", bufs=3)
                for t in range(T):
                    h = kh[t]
                    lhs = pTs[t][0:h, ts(mt, 128)]
                    nc.tensor.matmul(
                        psd[:, mt : mt + 1],
                        lhs,
                        ones_col[0:h, :],
                        start=(t == 0),
                        stop=(t == T - 1),
                    )
                    nc.tensor.matmul(
                        pso[:],
                        lhs,
                        kxn[0:h, ts(t, C)],
                        start=(t == 0),
                        stop=(t == T - 1),
                    )
                nc.vector.reciprocal(rd[:, mt : mt + 1], psd[:, mt : mt + 1])
                if mt == 0:
                    nc.scalar.mul(osb[:, ts(mt, C)], pso[:], rd[:, mt : mt + 1])
                else:
                    nc.vector.tensor_scalar(
                        osb[:, ts(mt, C)],
                        pso[:],
                        rd[:, mt : mt + 1],
                        None,
                        op0=ALU.mult,
                    )
            nc.scalar.dma_start(out_d[g, :, :], osb[:])

    nc.compile()
    return nc


def _prep_host(qx, kx, key_padding_mask, ln_q_g, ln_q_b, ln_k_g, ln_k_b, wq, wk):
    f32 = np.float32
    mask = np.asarray(key_padding_mask, bool)
    perm, plan = _plan(mask)

    # ---- Qg on host (exact restructure; see module docstring) ----
    qx32 = np.asarray(qx, f32).reshape(Bq, C)
    m = qx32.mean(axis=1, keepdims=True)
    v = ((qx32 - m) ** 2).mean(axis=1, keepdims=True)
    ln = (qx32 - m) / np.sqrt(v + EPS) * np.asarray(ln_q_g, f32)[None, :] + np.asarray(
        ln_q_b, f32
    )[None, :]
    qvec = ln.astype(np.float16).astype(f32) @ np.asarray(wq, f32).T
    qhat = qvec @ np.asarray(wk, f32)
    qg = qhat * (np.asarray(ln_k_g, f32) * (C ** -0.5))[None, :]
    qg = qg - qg.mean(axis=1, keepdims=True)  # fold k-side LN mean term
    qgT = np.ascontiguousarray(qg.T).astype(np.float16)  # [c, q]
    qgT_p = np.ascontiguousarray(
        qgT.reshape(4, 128, Bq).transpose(1, 0, 2).reshape(128, 4 * Bq)
    )

    # ---- per-row LN stats of kx on host; fold rsqrt(var) into kxt ----
    kx32 = np.asarray(kx, f32)  # [Bk, Nk, C]
    mk = kx32.mean(axis=-1, keepdims=True)
    vk = ((kx32 - mk) ** 2).mean(axis=-1)  # [Bk, Nk]
    r = 1.0 / np.sqrt(vk + EPS)
    bias = np.where(mask, MASK_NEG, 0.0).astype(f32)  # [Bk, Nk]

    kx16 = np.asarray(kx, np.float16)
    kxt_all = (kx32 * r[:, :, None]).astype(np.float16)  # r-scaled, [kb, n, c]
    ones = np.ones((128, 1), np.float16)
    in_maps = []
    for i in range(NCORES):
        batches = perm[:, i]
        kxs = kx16[batches]  # [BKPC, Nk, C]
        kxn = np.ascontiguousarray(
            kxs.reshape(BKPC, 4, 128, C).transpose(0, 2, 1, 3).reshape(BKPC, 128, 4 * C)
        )
        # kxt: [b][p][t][ci][n] = r*kx[b, t*128+n, ci*128+p], last tile
        # packed trimmed: cols (T-1)*512 + ci*nr + n
        a = kxt_all[batches].transpose(0, 2, 1)  # [b, c, n]
        full = (
            a.reshape(BKPC, 4, 128, 4, 128)  # [b, ci, p, t, n]
            .transpose(0, 2, 3, 1, 4)  # [b, p, t, ci, n]
            .reshape(BKPC, 128, 4 * 512)
        )
        kxt = np.ascontiguousarray(full)
        rbv = np.zeros((128, BKPC * 4), f32)
        bslab = bias[batches]  # [BKPC, Nk]
        for j in range(BKPC):
            rbv[:, j * 4 : j * 4 + 4] = bslab[j].reshape(4, 128).T
        in_maps.append(
            dict(
                qgT=qgT_p,
                ones=ones,
                rb=np.ascontiguousarray(rbv),
                kxn=kxn,
                kxt=np.ascontiguousarray(kxt),
            )
        )
    return in_maps, perm, plan


def _get_nc(plan):
    if _cache.get("plan") != plan:
        _cache["nc"] = _build_nc(plan)
        _cache["plan"] = plan
    return _cache["nc"]


def kernel(**inputs) -> np.ndarray:
    from concourse.bass_utils import run_bass_kernel_spmd

    in_maps, perm, plan = _prep_host(**inputs)
    nc = _get_nc(plan)
    res = run_bass_kernel_spmd(nc, in_maps, list(range(NCORES)))
    full = np.empty((Bq, Bk, C), np.float16)
    for i in range(NCORES):
        o = res.results[i]["out"]  # [BKPC, 128, 2C] packed
        o = o.reshape(BKPC, 128, 2, C).transpose(0, 2, 1, 3).reshape(BKPC, Bq, C)
        full[:, perm[:, i], :] = o.transpose(1, 0, 2)
    return np.ascontiguousarray(full)
